# revision 10
# baseline (speedup 1.0000x reference)
"""MiniBatchDiscrimination Trainium2 kernel.

reference:
    proj = x @ W.T                      # [512, 500] -> [512, 100, 5]
    l1[i,j,o] = sum_k |proj[i,o,k] - proj[j,o,k]|
    mbd[i,o]  = sum_j exp(-l1[i,j,o]) - 1
    out = concat([x, mbd], axis=1)      # [512, 1124]

Strategy (8 cores, shard i-rows of the BxB pairwise computation):
  - Host passes x.T (per-core column-rotated so that the core's 64 local
    rows sit in columns 0..63) and W.T with rows permuted k-major, so one
    SPMD program serves all cores with zero device-side core-id logic.
  - proj.T [500, 512] computed per core via PE matmul in bf16x2: inputs are
    split hi/lo and stacked along the contraction axis (hi@hi + lo@hi +
    hi@lo, K=3072), giving ~fp32 precision on the bf16 PE path.
  - Pairwise stage per local row i:
      absdiff tile t: A[p, j] = |proj.T[p, j] - proj.T[p, i]|  (bf16 out)
        - ScalarE:  Abs(-1 * projT + bias_col)      (fused, 1 elem/cyc)
        - VectorE:  tensor_scalar(sub, abs_max, 0)  (fused, 2 elem/cyc)
        split between both engines to balance load.
      k-reduce: PE matmul with a 0/1 selector S_t [125, 100] (bf16)
        contracting the partition axis, accumulating into PSUM [100, 512].
      exp + j-reduce: one ScalarE activation Exp(scale=-1) reading PSUM,
        with accum_out writing the free-axis sum straight into mbdT[:, i].
  - Host assembles: mbd = gather(mbdT).T - 1; out = [x | mbd].
"""

import sys

import numpy as np

sys.path.insert(0, "/opt/trn_rl_repo")

import ml_dtypes  # noqa: E402

import concourse.bacc as bacc  # noqa: E402
import concourse.bass as bass  # noqa: E402
import concourse.mybir as mybir  # noqa: E402
import concourse.tile as tile  # noqa: E402
from concourse.bass_utils import run_bass_kernel_spmd  # noqa: E402

B, IN, O, K = 512, 1024, 100, 5
OK = O * K  # 500
NCORES = 8
BL = B // NCORES  # 64 local rows per core
NT = 4  # proj.T partition tiles
PT = OK // NT  # 125 partitions per tile
NIN = (3 * IN) // 128  # 24 contraction chunks (hi/lo stacked)

F32 = mybir.dt.float32
BF16 = mybir.dt.bfloat16
U16 = mybir.dt.uint16
AF = mybir.ActivationFunctionType
ALU = mybir.AluOpType

# every ACT_EVERY-th absdiff tile runs on ScalarE, the rest on VectorE
ACT_EVERY = 6
GPS_EVERY = 10**9  # GpSimd rejects TensorScalar u16 — disabled
GSZ = 4  # i-rows per PSUM group; 2 groups pipeline across the 8 banks


def build():
    nc = bacc.Bacc("TRN2", target_bir_lowering=False)
    xc_d = nc.dram_tensor("xc", [3 * IN, B], BF16, kind="ExternalInput")
    wc_d = nc.dram_tensor("wc", [3 * IN, OK], BF16, kind="ExternalInput")
    sel = nc.dram_tensor("sel", [NT, PT, O], BF16, kind="ExternalInput")
    mbdT_d = nc.dram_tensor("mbdT", [O, BL], F32, kind="ExternalOutput")

    with tile.TileContext(nc) as tc:
        with (
            tc.tile_pool(name="pers", bufs=1) as pers,
            tc.tile_pool(name="io", bufs=NIN) as io,
            tc.tile_pool(name="work", bufs=12) as work,
            tc.tile_pool(name="esc", bufs=3) as esc,
            tc.tile_pool(name="ps", bufs=8, space="PSUM") as ps,
        ):
            # selector matrices (0/1), one per ok-tile
            s_sb = []
            for t in range(NT):
                s_t = pers.tile([PT, O], BF16, name=f"s{t}", tag=f"s{t}")
                nc.sync.dma_start(out=s_t[:], in_=sel[t])
                s_sb.append(s_t)

            # persistent proj.T tiles and the output accumulator
            projT = [
                pers.tile([PT, B], F32, name=f"projT{t}", tag=f"projT{t}")
                for t in range(NT)
            ]
            mbdT_sb = pers.tile([O, BL], F32, name="mbdT_sb", tag="mbdT_sb")

            # ---- proj phase: proj.T[p, j] = sum_in wc[in, p] * xc[in, j] ----
            pps = [ps.tile([PT, B], F32, name=f"pps{t}", tag="ps") for t in range(NT)]
            for c in range(NIN):
                x_c = io.tile([128, B], BF16, name=f"x{c}", tag="xc")
                nc.sync.dma_start(out=x_c[:], in_=xc_d[128 * c : 128 * (c + 1), :])
                w_c = io.tile([128, OK], BF16, name=f"w{c}", tag="wc")
                nc.sync.dma_start(out=w_c[:], in_=wc_d[128 * c : 128 * (c + 1), :])
                for t in range(NT):
                    nc.tensor.matmul(
                        pps[t][:],
                        lhsT=w_c[:, PT * t : PT * (t + 1)],
                        rhs=x_c[:],
                        start=(c == 0),
                        stop=(c == NIN - 1),
                    )
            for t in range(NT):
                nc.vector.tensor_copy(projT[t][:], pps[t][:])
            projTb = [
                pers.tile([PT, B], BF16, name=f"projTb{t}", tag=f"projTb{t}")
                for t in range(NT)
            ]
            for t in range(NT):
                nc.vector.tensor_copy(projTb[t][:], projT[t][:])

            # ---- pairwise phase ----
            for g0 in range(0, BL, GSZ):
                gis = range(g0, min(g0 + GSZ, BL))
                psums = {
                    i: ps.tile([O, B], F32, name=f"ps{i}", tag="ps") for i in gis
                }
                for t in range(NT):
                    for i in gis:
                        a = work.tile([PT, B], BF16, name=f"a{i}_{t}", tag="A")
                        col = projT[t][:, i : i + 1]
                        if (i * NT + t) % ACT_EVERY == 0:
                            nc.scalar.activation(
                                out=a[:],
                                in_=projT[t][:],
                                func=AF.Abs,
                                bias=col,
                                scale=-1.0,
                            )
                        else:
                            nc.vector.tensor_scalar(
                                a[:],
                                projTb[t][:],
                                col,
                                None,
                                op0=ALU.subtract,
                            )
                            and_eng = (
                                nc.gpsimd if (i * NT + t) % GPS_EVERY == 0 else nc.vector
                            )
                            and_eng.tensor_scalar(
                                a[:].bitcast(U16),
                                a[:].bitcast(U16),
                                0x7FFF,
                                None,
                                op0=ALU.bitwise_and,
                            )
                        nc.tensor.matmul(
                            psums[i][:],
                            lhsT=s_sb[t][:],
                            rhs=a[:],
                            start=(t == 0),
                            stop=(t == NT - 1),
                        )
                for i in gis:
                    e = esc.tile([O, B], F32, name=f"e{i}", tag="E")
                    nc.scalar.activation(
                        out=e[:],
                        in_=psums[i][:],
                        func=AF.Exp,
                        scale=-1.0,
                        accum_out=mbdT_sb[:, i : i + 1],
                    )

            nc.sync.dma_start(out=mbdT_d[:, :], in_=mbdT_sb[:])
    nc.compile()
    return nc


_CACHE = {}


def _build_cached():
    if "nc" not in _CACHE:
        _CACHE["nc"] = build()
    return _CACHE["nc"]


def _selector() -> np.ndarray:
    sel = np.zeros((NT, PT, O), np.float32)
    for t in range(NT):
        for p in range(PT):
            sel[t, p, (t * PT + p) % O] = 1.0
    return sel.astype(ml_dtypes.bfloat16)


def _split_bf16(a: np.ndarray):
    hi = a.astype(ml_dtypes.bfloat16)
    lo = (a - hi.astype(np.float32)).astype(ml_dtypes.bfloat16)
    return hi, lo


def make_in_maps(x: np.ndarray, W: np.ndarray):
    xT = np.ascontiguousarray(x.T.astype(np.float32))  # [IN, B]
    # k-major proj.T rows: row p corresponds to (o = p % O, k = p // O),
    # i.e. W row o*K + k
    perm = np.array([(p % O) * K + p // O for p in range(OK)], np.int64)
    wTk = np.ascontiguousarray(W.T.astype(np.float32)[:, perm])  # [IN, OK]
    # bf16x2: proj = xh@wh + xl@wh + xh@wl, stacked along contraction axis
    xh, xl = _split_bf16(xT)
    wh, wl = _split_bf16(wTk)
    xcat = np.concatenate([xh, xl, xh], axis=0)  # [3*IN, B]
    wcat = np.ascontiguousarray(np.concatenate([wh, wh, wl], axis=0))
    sel = _selector()
    in_maps = []
    for r in range(NCORES):
        in_maps.append(
            {
                "xc": np.ascontiguousarray(np.roll(xcat, -BL * r, axis=1)),
                "wc": wcat,
                "sel": sel,
            }
        )
    return in_maps


def run(x, W, trace=False, **kw):
    nc = _build_cached()
    in_maps = make_in_maps(x, W)
    return run_bass_kernel_spmd(
        nc, in_maps, core_ids=list(range(NCORES)), trace=trace, **kw
    )


def kernel(x: np.ndarray, W: np.ndarray) -> np.ndarray:
    x = np.asarray(x, np.float32)
    W = np.asarray(W, np.float32)
    res = run(x, W, trace=False)
    mbd = np.empty((B, O), np.float32)
    for r in range(NCORES):
        mbd[BL * r : BL * (r + 1), :] = res.results[r]["mbdT"].T
    mbd -= 1.0
    return np.concatenate([x, mbd], axis=1)


# revision 11
# speedup vs baseline: 1.3838x; 1.3838x over previous
"""MiniBatchDiscrimination Trainium2 kernel.

reference:
    proj = x @ W.T                      # [512, 500] -> [512, 100, 5]
    l1[i,j,o] = sum_k |proj[i,o,k] - proj[j,o,k]|
    mbd[i,o]  = sum_j exp(-l1[i,j,o]) - 1
    out = concat([x, mbd], axis=1)      # [512, 1124]

Strategy (8 cores, shard i-rows of the BxB pairwise computation):
  - Host passes x.T (per-core column-rotated so that the core's 64 local
    rows sit in columns 0..63) and W.T with rows permuted k-major, so one
    SPMD program serves all cores with zero device-side core-id logic.
  - Inputs are fp16: PE matmul runs full-rate single pass (~5e-4 precision),
    and the whole input load is 2 MB per core.
  - proj.T [500, 512] per core via PE matmul, kept as fp16 tiles [125, 512]
    x4 for the pairwise stage + small fp32 [125, 64] local-column blocks
    for bias/scalar operands.
  - Pairwise stage per local row i (A-quad [125, 4, 512] fp16):
      sub slice t:  ScalarE Abs(-projTb + bias_col)  (fused abs)
                or  VectorE tensor_scalar(sub)       (2x mode)
      one VectorE bitwise-AND 0x7FFF over the whole quad [125, 2048]
        clears fp16 sign bits -> |d| (idempotent on ACT-produced slices).
      k-reduce: PE matmul, 0/1 selector S_t [125, 100] fp16, contracting
        the partition axis, 4 slices accumulating into PSUM [100, 512].
      exp + j-reduce: one ScalarE Exp(scale=-1) reading PSUM, accum_out
        writes the free-axis sum straight into mbdT[:, i].
  - Host assembles: mbd = gather(mbdT).T - 1; out = [x | mbd].
"""

import sys

import numpy as np

sys.path.insert(0, "/opt/trn_rl_repo")

import concourse.bacc as bacc  # noqa: E402
import concourse.mybir as mybir  # noqa: E402
import concourse.tile as tile  # noqa: E402
from concourse.bass_utils import run_bass_kernel_spmd  # noqa: E402

B, IN, O, K = 512, 1024, 100, 5
OK = O * K  # 500
NCORES = 8
BL = B // NCORES  # 64 local rows per core
NT = 4  # proj.T partition tiles
PT = OK // NT  # 125 partitions per tile
NIN = IN // 128  # 8 contraction chunks

F32 = mybir.dt.float32
F16 = mybir.dt.float16
U16 = mybir.dt.uint16
AF = mybir.ActivationFunctionType
ALU = mybir.AluOpType

# every ACT_EVERY-th absdiff slice runs on ScalarE, the rest on VectorE
ACT_EVERY = 4
GSZ = 4  # i-rows per PSUM group; 2 groups pipeline across the 8 banks


def build():
    nc = bacc.Bacc("TRN2", target_bir_lowering=False)
    xT_d = nc.dram_tensor("xT", [IN, B], F16, kind="ExternalInput")
    wT_d = nc.dram_tensor("wT", [IN, OK], F16, kind="ExternalInput")
    sel = nc.dram_tensor("sel", [NT, PT, O], F16, kind="ExternalInput")
    mbdT_d = nc.dram_tensor("mbdT", [O, BL], F32, kind="ExternalOutput")

    with tile.TileContext(nc) as tc:
        with (
            tc.tile_pool(name="pers", bufs=1) as pers,
            tc.tile_pool(name="io", bufs=NIN) as io,
            tc.tile_pool(name="work", bufs=3) as work,
            tc.tile_pool(name="esc", bufs=3) as esc,
            tc.tile_pool(name="ps", bufs=8, space="PSUM") as ps,
        ):
            # selector matrices (0/1), one per ok-tile
            s_sb = []
            for t in range(NT):
                s_t = pers.tile([PT, O], F16, name=f"s{t}", tag=f"s{t}")
                nc.sync.dma_start(out=s_t[:], in_=sel[t])
                s_sb.append(s_t)

            # persistent proj.T tiles (fp16 full + fp32 local cols) and output
            projTb = [
                pers.tile([PT, B], F16, name=f"projTb{t}", tag=f"projTb{t}")
                for t in range(NT)
            ]
            projL = [
                pers.tile([PT, BL], F32, name=f"projL{t}", tag=f"projL{t}")
                for t in range(NT)
            ]
            mbdT_sb = pers.tile([O, BL], F32, name="mbdT_sb", tag="mbdT_sb")

            # ---- proj phase: proj.T[p, j] = sum_in wT[in, p] * xT[in, j] ----
            pps = [ps.tile([PT, B], F32, name=f"pps{t}", tag="ps") for t in range(NT)]
            for c in range(NIN):
                x_c = io.tile([128, B], F16, name=f"x{c}", tag="xc")
                nc.sync.dma_start(out=x_c[:], in_=xT_d[128 * c : 128 * (c + 1), :])
                w_c = io.tile([128, OK], F16, name=f"w{c}", tag="wc")
                nc.sync.dma_start(out=w_c[:], in_=wT_d[128 * c : 128 * (c + 1), :])
                for t in range(NT):
                    nc.tensor.matmul(
                        pps[t][:],
                        lhsT=w_c[:, PT * t : PT * (t + 1)],
                        rhs=x_c[:],
                        start=(c == 0),
                        stop=(c == NIN - 1),
                    )
            for t in range(NT):
                nc.vector.tensor_copy(projTb[t][:], pps[t][:])
                nc.scalar.copy(projL[t][:], pps[t][:, :BL])

            # ---- pairwise phase ----
            for g0 in range(0, BL, GSZ):
                gis = range(g0, min(g0 + GSZ, BL))
                psums = {
                    i: ps.tile([O, B], F32, name=f"ps{i}", tag="ps") for i in gis
                }
                for i in gis:
                    aq = work.tile([PT, NT, B], F16, name=f"a{i}", tag="A")
                    for t in range(NT):
                        col = projL[t][:, i : i + 1]
                        if (i * NT + t) % ACT_EVERY == 0:
                            nc.scalar.activation(
                                out=aq[:, t, :],
                                in_=projTb[t][:],
                                func=AF.Abs,
                                bias=col,
                                scale=-1.0,
                            )
                        else:
                            nc.vector.tensor_scalar(
                                aq[:, t, :],
                                projTb[t][:],
                                col,
                                None,
                                op0=ALU.subtract,
                            )
                    nc.vector.tensor_scalar(
                        aq[:].bitcast(U16),
                        aq[:].bitcast(U16),
                        0x7FFF,
                        None,
                        op0=ALU.bitwise_and,
                    )
                    for t in range(NT):
                        nc.tensor.matmul(
                            psums[i][:],
                            lhsT=s_sb[t][:],
                            rhs=aq[:, t, :],
                            start=(t == 0),
                            stop=(t == NT - 1),
                        )
                for i in gis:
                    e = esc.tile([O, B], F16, name=f"e{i}", tag="E")
                    nc.scalar.activation(
                        out=e[:],
                        in_=psums[i][:],
                        func=AF.Exp,
                        scale=-1.0,
                        accum_out=mbdT_sb[:, i : i + 1],
                    )

            nc.sync.dma_start(out=mbdT_d[:, :], in_=mbdT_sb[:])
    nc.compile()
    return nc


_CACHE = {}


def _build_cached():
    if "nc" not in _CACHE:
        _CACHE["nc"] = build()
    return _CACHE["nc"]


def _selector() -> np.ndarray:
    sel = np.zeros((NT, PT, O), np.float32)
    for t in range(NT):
        for p in range(PT):
            sel[t, p, (t * PT + p) % O] = 1.0
    return sel.astype(np.float16)


def make_in_maps(x: np.ndarray, W: np.ndarray):
    xT = np.ascontiguousarray(x.T.astype(np.float16))  # [IN, B]
    # k-major proj.T rows: row p corresponds to (o = p % O, k = p // O),
    # i.e. W row o*K + k
    perm = np.array([(p % O) * K + p // O for p in range(OK)], np.int64)
    wTk = np.ascontiguousarray(W.T.astype(np.float16)[:, perm])  # [IN, OK]
    sel = _selector()
    in_maps = []
    for r in range(NCORES):
        in_maps.append(
            {
                "xT": np.ascontiguousarray(np.roll(xT, -BL * r, axis=1)),
                "wT": wTk,
                "sel": sel,
            }
        )
    return in_maps


def run(x, W, trace=False, **kw):
    nc = _build_cached()
    in_maps = make_in_maps(x, W)
    return run_bass_kernel_spmd(
        nc, in_maps, core_ids=list(range(NCORES)), trace=trace, **kw
    )


def kernel(x: np.ndarray, W: np.ndarray) -> np.ndarray:
    x = np.asarray(x, np.float32)
    W = np.asarray(W, np.float32)
    res = run(x, W, trace=False)
    mbd = np.empty((B, O), np.float32)
    for r in range(NCORES):
        mbd[BL * r : BL * (r + 1), :] = res.results[r]["mbdT"].T
    mbd -= 1.0
    return np.concatenate([x, mbd], axis=1)
